# revision 16
# baseline (speedup 1.0000x reference)
"""CenterLoss Trainium2 kernel (fp8 DoubleRow, on-device squaring).

Full inputs:
  ep_mask_embed    (8, 4096, 256) f32
  ep_mask          (8, 1, 1024, 1024) f32
  query_mask_embed (8, 4096, 256) f32
  query_mask       (8, 1, 1024, 1024) f32
Output: (3,) f32 = [mean(center_loss), mean(pos_loss), mean(neg_loss)]

Sharding: data-parallel, one batch sample per NeuronCore (8 cores).

Per sample the loss reduces to epw = [m;1-m]^T ep, qw = [m;1-m]^T q,
qsqw = [m;1-m]^T q^2 plus mask counts; everything downstream is ~50
scalar flops done on host from those statistics (where the batch mean
already happens).

Trace-driven structure (v7):
  - HBM traffic 2MB/core (not 3MB): q^2 is squared on-device instead
    of host-precomputed+streamed.  DVE takes cols [0:1792) of each
    512KB chunk, ACT the rest (balances the errata-adjusted rates
    (151+x)/0.96 vs (224+4096-x)/1.2); exactly one op per engine per
    chunk -- more ops pay the DVE inter-op DRAIN (~op_dur-266ns).
  - All DMAs ride the SYNC HWDGE ring in consumption order (lw, q0,
    q1, ep0, ep1).  lw is padded to 512B/partition: sub-512B
    descriptors pay an SDMA read-modify-write penalty that was
    measured to stall the whole stream queue.  The ACT ring stays
    free; a data-free dummy Square forces its ~2.7us ACT_TABLE_LOAD
    into the DMA ramp (otherwise walrus attaches it to the first real
    square's data-gated bundle).
  - PE warm-up: zero matmuls up-front and between the front bursts
    keep the PE busy through the release-paced phase so the HAM clock
    gate (4/8 -> 8/8, ~3.4us window) flips early; cold matmuls run
    213ns vs 109ns warm.
  - fp8 re-rounding bias of q^2 (squares of fp8 grid points land
    systematically off RNE midpoints; measured b(s) is log-periodic
    and mostly negative) is cancelled by block dither: the four
    (chunk x engine-branch) blocks use scales whose measured biases
    cancel (7*b(.8125)+9*b(.9375)+7*b(1.625)+9*b(1.875) ~ 0, +0.001%
    on synthetic N(0,1)); scales are immediates, their reciprocals
    are EXACT fp8 grid points folded into dedicated qsq weight
    planes, so the unmix is exact.
  - Single out-DMA for all three stat sections.
"""

import numpy as np
import ml_dtypes
from contextlib import ExitStack

import concourse.bass as bass
import concourse.bacc as bacc
import concourse.tile as tile
from concourse import mybir
from concourse.bass_utils import run_bass_kernel_spmd

F32 = mybir.dt.float32
F8 = mybir.dt.float8e4
NP_F8 = ml_dtypes.float8_e4m3fn

P = 128          # partitions
N_TOK = 4096     # tokens per sample (64*64 patches)
C = 256          # channels
T = 16           # tokens per partition per chunk (4KB fp8 descriptor)
DC = P * T       # tokens per chunk (2048)
N_DC = N_TOK // DC   # 2 chunks
NPC = T // 2     # parity-pairs (pieces) per chunk: 8
B = 8            # batch == n cores
PATCH = 16
SQ_SPLIT = 1792  # DVE squares chunk cols [0:1792) = local tokens 0-6,
                 # ACT the rest (token-aligned: 1792 = 7*256)
NM_PLANES = 6    # weight planes: q_pos,q_neg,ep_pos,ep_neg,qsq_pos,qsq_neg
LW_COLS = 2 * 256    # two ks planes of 256 cols (96 used per plane;
                     # padded to 512B/partition descriptors)
N_WARM = 10          # up-front PE warm-up matmuls
N_WARM_MID = 2       # warm-ups between front bursts (fill release gaps)

# block dither: inv-scale per (chunk, branch); all EXACT fp8e4m3 grid
# points; measured per-scale biases cancel in the 7:9 col-split ratio.
_INV_BLOCK = {(0, "dve"): 0.8125, (0, "act"): 0.9375,
              (1, "dve"): 1.625, (1, "act"): 1.875}
_S_DVE = [1.0 / _INV_BLOCK[(0, "dve")], 1.0 / _INV_BLOCK[(1, "dve")]]
_SQRT_S_ACT = [float(np.sqrt(1.0 / _INV_BLOCK[(0, "act")])),
               float(np.sqrt(1.0 / _INV_BLOCK[(1, "act")]))]

_CACHE = {}


def _build():
    """Build the per-core Bass program (identical on all cores)."""
    nc = bacc.Bacc("TRN2", target_bir_lowering=False, debug=False)

    ep8 = nc.dram_tensor("ep8", [N_TOK, C], F8, kind="ExternalInput").ap()
    q8 = nc.dram_tensor("q8", [N_TOK, C], F8, kind="ExternalInput").ap()
    # host-packed DoubleRow mask weights.  The dual-fp8 ldweights ISA
    # check needs the dual-row AP dim to have num_elem==2 and a step
    # that is a multiple of 16 elements, so the two ks sub-rows live in
    # separate 256-col planes: col = 256*ks + 6*jj + m,
    # m in (q_pos, q_neg, ep_pos, ep_neg, qsq_pos, qsq_neg),
    # token = 2048*(jj//8) + 16*p + 2*(jj%8) + ks
    lw = nc.dram_tensor("lw", [P, LW_COLS], F8, kind="ExternalInput").ap()
    # [epw | qw | qsqw], rows = (pos, neg)
    out = nc.dram_tensor("out", [2, 3 * C], F32, kind="ExternalOutput").ap()

    DR = mybir.MatmulPerfMode.DoubleRow

    with tile.TileContext(nc) as tc, ExitStack() as ctx:
        const_pool = ctx.enter_context(tc.tile_pool(name="const", bufs=1))
        x_pool = ctx.enter_context(tc.tile_pool(name="x_pool", bufs=1))
        sq_pool = ctx.enter_context(tc.tile_pool(name="sq_pool", bufs=1))
        psum_pool = ctx.enter_context(
            tc.tile_pool(name="psum", bufs=1, space=bass.MemorySpace.PSUM)
        )
        fin_pool = ctx.enter_context(tc.tile_pool(name="fin", bufs=1))

        # PE warm-up scratch (zeros) + warm-up matmul helper
        warm = const_pool.tile([P, C], F8, name="warm", tag="warm")
        nc.gpsimd.memset(warm[:], 0)
        warm_ps = psum_pool.tile([P, C], F32, name="warm_ps", tag="warm_ps")

        def warmup(n):
            for _ in range(n):
                nc.tensor.matmul(
                    warm_ps[:], warm[:, 0:P], warm[:], start=True, stop=True
                )

        warmup(N_WARM)

        # Data-free dummy Square: forces the ACT_TABLE_LOAD to run
        # during the DMA ramp instead of attached to the first real
        # (data-gated) square.
        act_dummy = const_pool.tile([P, 8], F8, name="act_dummy",
                                    tag="act_dummy")
        nc.scalar.square(act_dummy[:], warm[:, 0:8])

        # DMAs in consumption order on the sync ring; each unit is its
        # own tile so its completion releases consumers independently
        # (Tile tracks deps at whole-tile granularity).
        lw_t = const_pool.tile([P, LW_COLS], F8, name="lw_t", tag="lw_t")
        nc.sync.dma_start(out=lw_t[:], in_=lw[:])

        X = {}
        for nm, src in (("q", q8), ("ep", ep8)):
            for i in range(N_DC):
                t_ = x_pool.tile([P, T * C], F8, name=f"x{nm}{i}",
                                 tag=f"x{nm}{i}")
                X[(nm, i)] = t_
        for nm, i in (("q", 0), ("q", 1), ("ep", 0), ("ep", 1)):
            src = q8 if nm == "q" else ep8
            nc.sync.dma_start(
                out=X[(nm, i)][:],
                in_=src[i * DC:(i + 1) * DC, :].rearrange(
                    "(p t) c -> p (t c)", t=T),
            )

        # On-device s*q^2 (fp8 in/out, fp32 internal): one DVE op and
        # one ACT op per chunk, per-block immediate dither scales.
        SQ = {}
        for i in range(N_DC):
            sq = sq_pool.tile([P, T * C], F8, name=f"sq{i}", tag=f"sq{i}")
            qt = X[("q", i)]
            nc.vector.scalar_tensor_tensor(
                sq[:, 0:SQ_SPLIT],
                qt[:, 0:SQ_SPLIT],
                float(_S_DVE[i]),
                qt[:, 0:SQ_SPLIT],
                mybir.AluOpType.mult,
                mybir.AluOpType.mult,
            )
            nc.scalar.activation(
                sq[:, SQ_SPLIT:T * C],
                qt[:, SQ_SPLIT:T * C],
                mybir.ActivationFunctionType.Square,
                scale=_SQRT_S_ACT[i],
            )
            SQ[("qsq", i)] = sq

        psum = {
            nm: psum_pool.tile([2, C], F32, name=f"ps_{nm}", tag=f"ps_{nm}")
            for nm in ("ep", "q", "qsq")
        }

        fin = fin_pool.tile([2, 3 * C], F32, name="fin", tag="fin")
        SEC = {"ep": 0, "q": 1, "qsq": 2}
        WOFF = {"q": 0, "ep": 2, "qsq": 4}

        # PE bursts in expected data-availability order; warm-ups fill
        # the release-paced gaps between the front bursts.
        bursts = [
            ("q", 0), ("warm", None), ("q", 1), ("warm", None),
            ("qsq", 0), ("ep", 0), ("qsq", 1), ("ep", 1),
        ]
        for nm, i in bursts:
            if nm == "warm":
                warmup(N_WARM_MID)
                continue
            src_t = SQ[(nm, i)] if nm == "qsq" else X[(nm, i)]
            for j in range(NPC):
                jj = NPC * i + j
                off = NM_PLANES * jj + WOFF[nm]
                w = lw_t[:].rearrange(
                    "p (k c) -> p k c", k=2)[:, :, off:off + 2]
                rhs = src_t[:, 512 * j:512 * (j + 1)].rearrange(
                    "p (k c) -> p k c", k=2)
                nc.tensor.matmul(
                    psum[nm][:], w, rhs,
                    start=(i == 0 and j == 0),
                    stop=(i == N_DC - 1 and j == NPC - 1),
                    perf_mode=DR,
                )
            if i == N_DC - 1:
                s = SEC[nm]
                # last chain (ep) ships via the by-then-idle DVE; the
                # earlier two via ACT (its squares are done by then).
                fsec = fin[:, s * C:(s + 1) * C]
                if nm == "ep":
                    nc.vector.tensor_copy(fsec, psum[nm][:])
                else:
                    nc.scalar.copy(fsec, psum[nm][:])

        # single out-DMA for all three sections
        nc.sync.dma_start(out=out[:], in_=fin[:])

    nc.compile()
    return nc


def get_nc():
    if "nc" not in _CACHE:
        _CACHE["nc"] = _build()
    return _CACHE["nc"]


# token index per (partition, piece jj, ks): DoubleRow weight layout
_PG = np.arange(P)[:, None, None]
_JJ = np.arange(N_DC * NPC)[None, :, None]
_KS = np.arange(2)[None, None, :]
_TOK = (DC * (_JJ // NPC) + T * _PG + 2 * (_JJ % NPC) + _KS)  # [128, 16, 2]

# per-(jj, ks) block inverse dither scale (same for every partition)
_L_IDX = 2 * (_JJ % NPC) + _KS          # local token index 0..15
_CHUNK = _JJ // NPC                     # chunk 0/1
_INV_TOK = np.where(
    _CHUNK == 0,
    np.where(_L_IDX < SQ_SPLIT // C, _INV_BLOCK[(0, "dve")],
             _INV_BLOCK[(0, "act")]),
    np.where(_L_IDX < SQ_SPLIT // C, _INV_BLOCK[(1, "dve")],
             _INV_BLOCK[(1, "act")]),
)  # [1, 16, 2] broadcastable over partitions


def _mask_ds(mask_b):
    """Downsample one sample's mask (nearest, stride 16) -> (4096,) f64."""
    return mask_b[0, ::PATCH, ::PATCH].reshape(-1).astype(np.float64)


def make_in_maps(ep_mask_embed, ep_mask, query_mask_embed, query_mask):
    in_maps, counts = [], []
    for b in range(B):
        em = _mask_ds(ep_mask[b])
        qm = _mask_ds(query_mask[b])
        et = em[_TOK]  # [128, 16, 2] = (p, jj, ks)
        qt = qm[_TOK]
        # weight planes: q_pos, q_neg, ep_pos, ep_neg, qsq_pos/s, qsq_neg/s
        L = np.stack([qt, 1.0 - qt, et, 1.0 - et,
                      qt * _INV_TOK, (1.0 - qt) * _INV_TOK],
                     axis=-1)  # [p,jj,ks,m]
        lw_b = np.zeros((P, 2, LW_COLS // 2), dtype=np.float64)
        # col = 256*ks + 6*jj + m
        lw_b[:, :, :NM_PLANES * N_DC * NPC] = (
            L.transpose(0, 2, 1, 3).reshape(P, 2, NM_PLANES * N_DC * NPC))
        in_maps.append({
            "ep8": np.ascontiguousarray(ep_mask_embed[b]).astype(NP_F8),
            "q8": np.ascontiguousarray(query_mask_embed[b]).astype(NP_F8),
            "lw": lw_b.reshape(P, LW_COLS).astype(NP_F8),
        })
        counts.append((em.sum(), (1.0 - em).sum(), qm.sum(), (1.0 - qm).sum()))
    return in_maps, counts


def finalize(per_core, counts):
    """per_core: list of 8 arrays [2, 768] (epw|qw|qsqw) -> full (3,)."""
    pos = np.zeros(B)
    neg = np.zeros(B)
    for b in range(B):
        st = np.asarray(per_core[b]).astype(np.float64)
        n_pe, n_ne, n_pq, n_nq = counts[b]
        epw, qw, qsq = st[:, 0:C], st[:, C:2 * C], st[:, 2 * C:3 * C]
        pc = epw[0] / (n_pe + 0.1)
        ncen = epw[1] / (n_ne + 0.1)
        pn = qsq[0].sum() - 2.0 * (pc @ qw[0]) + n_pq * (pc @ pc)
        nn = qsq[1].sum() - 2.0 * (ncen @ qw[1]) + n_nq * (ncen @ ncen)
        pos[b] = pn / (max(n_pq, 1.0) * C) if n_pq > 0 else 0.0
        neg[b] = nn / (max(n_nq, 1.0) * C) if n_nq > 0 else 0.0
    return np.array(
        [(pos + neg).mean(), pos.mean(), neg.mean()], dtype=np.float32
    )


def kernel(ep_mask_embed, ep_mask, query_mask_embed, query_mask):
    ep_mask_embed = np.asarray(ep_mask_embed, dtype=np.float32)
    ep_mask = np.asarray(ep_mask, dtype=np.float32)
    query_mask_embed = np.asarray(query_mask_embed, dtype=np.float32)
    query_mask = np.asarray(query_mask, dtype=np.float32)

    nc = get_nc()
    in_maps, counts = make_in_maps(
        ep_mask_embed, ep_mask, query_mask_embed, query_mask)
    # First execution after device bring-up has been observed to return
    # garbage once; retry on non-finite results.
    for _ in range(3):
        res = run_bass_kernel_spmd(nc, in_maps, list(range(B)))
        result = finalize([r["out"] for r in res.results], counts)
        if np.all(np.isfinite(result)):
            return result
    return result


# revision 24
# speedup vs baseline: 1.0826x; 1.0826x over previous
"""CenterLoss Trainium2 kernel (fp8 DoubleRow, on-device squaring).

Full inputs:
  ep_mask_embed    (8, 4096, 256) f32
  ep_mask          (8, 1, 1024, 1024) f32
  query_mask_embed (8, 4096, 256) f32
  query_mask       (8, 1, 1024, 1024) f32
Output: (3,) f32 = [mean(center_loss), mean(pos_loss), mean(neg_loss)]

Sharding: data-parallel, one batch sample per NeuronCore (8 cores).

Per sample the loss reduces to epw = [m;1-m]^T ep, qw = [m;1-m]^T q,
qsqw = [m;1-m]^T q^2 plus mask counts; everything downstream is ~50
scalar flops done on host from those statistics (where the batch mean
already happens).

Trace-driven structure (v8):
  - HBM traffic 2MB/core (not 3MB): q^2 is squared on-device instead
    of host-precomputed+streamed, split across the otherwise idle
    DVE (tokens 0-5) and ACT (tokens 6-15) of each 512KB chunk;
    exactly one op per engine per chunk -- more ops pay the DVE
    inter-op DRAIN (~op_dur-266ns).  (GPSIMD elementwise ops do not
    lower in this walrus build.)
  - All DMAs ride the SYNC HWDGE ring in consumption order (lw, q0,
    q1, ep0, ep1).  lw is padded to 512B/partition: sub-512B
    descriptors pay an SDMA read-modify-write penalty that was
    measured to stall the whole stream queue.  The ACT ring stays
    free; a data-free dummy Square forces its ~2.7us ACT_TABLE_LOAD
    into the DMA ramp (otherwise walrus attaches it to the first real
    square's data-gated bundle).
  - PE warm-up: zero matmuls up-front and between the front bursts
    keep the PE busy through the release-paced phase so the HAM clock
    gate (4/8 -> 8/8, ~3.4us window) flips early; cold matmuls run
    213ns vs 109ns warm.
  - fp8 re-rounding bias of q^2 (squares of fp8 grid points land
    systematically off RNE midpoints; measured b(s) is log-periodic
    and mostly negative) is cancelled by block dither: the four
    (chunk x engine-branch) blocks use scales whose measured biases
    cancel (7*b(.8125)+9*b(.9375)+7*b(1.625)+9*b(1.875) ~ 0, +0.001%
    on synthetic N(0,1)); scales are immediates, their reciprocals
    are EXACT fp8 grid points folded into dedicated qsq weight
    planes, so the unmix is exact.
  - Single out-DMA for all three stat sections.
"""

import numpy as np
import ml_dtypes
from contextlib import ExitStack

import concourse.bass as bass
import concourse.bacc as bacc
import concourse.tile as tile
from concourse import mybir
from concourse.bass_utils import run_bass_kernel_spmd

F32 = mybir.dt.float32
F8 = mybir.dt.float8e4
NP_F8 = ml_dtypes.float8_e4m3fn

P = 128          # partitions
N_TOK = 4096     # tokens per sample (64*64 patches)
C = 256          # channels
T = 16           # tokens per partition per chunk (4KB fp8 descriptor)
DC = P * T       # tokens per chunk (2048)
N_DC = N_TOK // DC   # 2 chunks
NPC = T // 2     # parity-pairs (pieces) per chunk: 8
B = 8            # batch == n cores
PATCH = 16
SQ_DVE = 1536    # DVE squares chunk cols [0:1536) = local tokens 0-5,
                 # ACT the rest (token-aligned multiple of 256)
NM_PLANES = 6    # weight planes: q_pos,q_neg,ep_pos,ep_neg,qsq_pos,qsq_neg
LW_COLS = 2 * 256    # two ks planes of 256 cols (96 used per plane;
                     # padded to 512B/partition descriptors)
N_WARM = 10          # up-front PE warm-up matmuls
N_WARM_MID = 2       # warm-ups between front bursts (fill release gaps)

# Block dither for the fp8 re-rounding bias of q^2 (measured b(s):
# b(1)=-0.693%, b(1/0.9375)=+0.737%, b(1/0.6875)=+0.121%): DVE runs
# plain (tensor_mul is ~15% faster than scalar_tensor_tensor), ACT
# carries the compensating scales 1/0.9375 then 1/0.6875 per chunk;
# with the 6:10 token split the weighted biases cancel (+0.008% on
# synthetic N(0,1)).  Both inv-scales are EXACT fp8e4m3 grid points
# folded into the qsq weight planes, so the unmix is exact.
_INV_ACT = [0.9375, 0.6875]
_SQRT_S_ACT = [float(np.sqrt(1.0 / v)) for v in _INV_ACT]

_CACHE = {}


def _build():
    """Build the per-core Bass program (identical on all cores)."""
    nc = bacc.Bacc("TRN2", target_bir_lowering=False, debug=False)

    ep8 = nc.dram_tensor("ep8", [N_TOK, C], F8, kind="ExternalInput").ap()
    q8 = nc.dram_tensor("q8", [N_TOK, C], F8, kind="ExternalInput").ap()
    # host-packed DoubleRow mask weights.  The dual-fp8 ldweights ISA
    # check needs the dual-row AP dim to have num_elem==2 and a step
    # that is a multiple of 16 elements, so the two ks sub-rows live in
    # separate 256-col planes: col = 256*ks + 6*jj + m,
    # m in (q_pos, q_neg, ep_pos, ep_neg, qsq_pos, qsq_neg),
    # token = 2048*(jj//8) + 16*p + 2*(jj%8) + ks
    lw = nc.dram_tensor("lw", [P, LW_COLS], F8, kind="ExternalInput").ap()
    # [epw | qw | qsqw], rows = (pos, neg)
    out = nc.dram_tensor("out", [2, 3 * C], F32, kind="ExternalOutput").ap()

    DR = mybir.MatmulPerfMode.DoubleRow

    with tile.TileContext(nc) as tc, ExitStack() as ctx:
        const_pool = ctx.enter_context(tc.tile_pool(name="const", bufs=1))
        x_pool = ctx.enter_context(tc.tile_pool(name="x_pool", bufs=1))
        sq_pool = ctx.enter_context(tc.tile_pool(name="sq_pool", bufs=1))
        psum_pool = ctx.enter_context(
            tc.tile_pool(name="psum", bufs=1, space=bass.MemorySpace.PSUM)
        )
        fin_pool = ctx.enter_context(tc.tile_pool(name="fin", bufs=1))

        # PE warm-up scratch (zeros) + warm-up matmul helper
        warm = const_pool.tile([P, C], F8, name="warm", tag="warm")
        nc.gpsimd.memset(warm[:], 0)
        warm_ps = psum_pool.tile([P, C], F32, name="warm_ps", tag="warm_ps")

        def warmup(n):
            for _ in range(n):
                nc.tensor.matmul(
                    warm_ps[:], warm[:, 0:P], warm[:], start=True, stop=True
                )

        warmup(N_WARM)

        # Data-free dummy Square: forces the ACT_TABLE_LOAD to run
        # during the DMA ramp instead of attached to the first real
        # (data-gated) square.
        act_dummy = const_pool.tile([P, 8], F8, name="act_dummy",
                                    tag="act_dummy")
        nc.scalar.square(act_dummy[:], warm[:, 0:8])

        # DMAs in consumption order on the sync ring; each unit is its
        # own tile so its completion releases consumers independently
        # (Tile tracks deps at whole-tile granularity).
        lw_t = const_pool.tile([P, LW_COLS], F8, name="lw_t", tag="lw_t")
        nc.sync.dma_start(out=lw_t[:], in_=lw[:])

        X = {}
        for nm, src in (("q", q8), ("ep", ep8)):
            for i in range(N_DC):
                t_ = x_pool.tile([P, T * C], F8, name=f"x{nm}{i}",
                                 tag=f"x{nm}{i}")
                X[(nm, i)] = t_
        for nm, i in (("q", 0), ("q", 1), ("ep", 0), ("ep", 1)):
            src = q8 if nm == "q" else ep8
            nc.sync.dma_start(
                out=X[(nm, i)][:],
                in_=src[i * DC:(i + 1) * DC, :].rearrange(
                    "(p t) c -> p (t c)", t=T),
            )

        # On-device s*q^2 (fp8 in/out, fp32 internal): one DVE op and
        # one ACT op per chunk; the ACT ops carry the dither scales.
        SQ = {}
        for i in range(N_DC):
            sq = sq_pool.tile([P, T * C], F8, name=f"sq{i}", tag=f"sq{i}")
            qt = X[("q", i)]
            nc.vector.tensor_mul(
                sq[:, 0:SQ_DVE], qt[:, 0:SQ_DVE], qt[:, 0:SQ_DVE])
            nc.scalar.activation(
                sq[:, SQ_DVE:T * C],
                qt[:, SQ_DVE:T * C],
                mybir.ActivationFunctionType.Square,
                scale=_SQRT_S_ACT[i],
            )
            SQ[("qsq", i)] = sq

        psum = {
            nm: psum_pool.tile([2, C], F32, name=f"ps_{nm}", tag=f"ps_{nm}")
            for nm in ("ep", "q", "qsq")
        }

        fin = fin_pool.tile([2, 3 * C], F32, name="fin", tag="fin")
        SEC = {"ep": 0, "q": 1, "qsq": 2}
        WOFF = {"q": 0, "ep": 2, "qsq": 4}

        # PE bursts in expected data-availability order; warm-ups fill
        # the release-paced gaps between the front bursts.
        bursts = [
            ("q", 0), ("warm", None), ("q", 1), ("warm", None),
            ("qsq", 0), ("ep", 0), ("qsq", 1), ("ep", 1),
        ]
        for nm, i in bursts:
            if nm == "warm":
                warmup(N_WARM_MID)
                continue
            src_t = SQ[(nm, i)] if nm == "qsq" else X[(nm, i)]
            for j in range(NPC):
                jj = NPC * i + j
                off = NM_PLANES * jj + WOFF[nm]
                w = lw_t[:].rearrange(
                    "p (k c) -> p k c", k=2)[:, :, off:off + 2]
                rhs = src_t[:, 512 * j:512 * (j + 1)].rearrange(
                    "p (k c) -> p k c", k=2)
                nc.tensor.matmul(
                    psum[nm][:], w, rhs,
                    start=(i == 0 and j == 0),
                    stop=(i == N_DC - 1 and j == NPC - 1),
                    perf_mode=DR,
                )
            if i == N_DC - 1:
                s = SEC[nm]
                # last chain (ep) ships via the by-then-idle DVE; the
                # earlier two via ACT (its squares are done by then).
                fsec = fin[:, s * C:(s + 1) * C]
                if nm == "ep":
                    nc.vector.tensor_copy(fsec, psum[nm][:])
                else:
                    nc.scalar.copy(fsec, psum[nm][:])

        # single out-DMA for all three sections
        nc.sync.dma_start(out=out[:], in_=fin[:])

    nc.compile()
    return nc


def get_nc():
    if "nc" not in _CACHE:
        _CACHE["nc"] = _build()
    return _CACHE["nc"]


# token index per (partition, piece jj, ks): DoubleRow weight layout
_PG = np.arange(P)[:, None, None]
_JJ = np.arange(N_DC * NPC)[None, :, None]
_KS = np.arange(2)[None, None, :]
_TOK = (DC * (_JJ // NPC) + T * _PG + 2 * (_JJ % NPC) + _KS)  # [128, 16, 2]

# per-(jj, ks) block inverse dither scale (same for every partition)
_L_IDX = 2 * (_JJ % NPC) + _KS          # local token index 0..15
_CHUNK = _JJ // NPC                     # chunk 0/1
_INV_TOK = np.where(
    _L_IDX < SQ_DVE // C, 1.0,
    np.where(_CHUNK == 0, _INV_ACT[0], _INV_ACT[1]),
)  # [1, 16, 2] broadcastable over partitions


def _mask_ds(mask_b):
    """Downsample one sample's mask (nearest, stride 16) -> (4096,) f64."""
    return mask_b[0, ::PATCH, ::PATCH].reshape(-1).astype(np.float64)


def make_in_maps(ep_mask_embed, ep_mask, query_mask_embed, query_mask):
    in_maps, counts = [], []
    for b in range(B):
        em = _mask_ds(ep_mask[b])
        qm = _mask_ds(query_mask[b])
        et = em[_TOK]  # [128, 16, 2] = (p, jj, ks)
        qt = qm[_TOK]
        # weight planes: q_pos, q_neg, ep_pos, ep_neg, qsq_pos/s, qsq_neg/s
        L = np.stack([qt, 1.0 - qt, et, 1.0 - et,
                      qt * _INV_TOK, (1.0 - qt) * _INV_TOK],
                     axis=-1)  # [p,jj,ks,m]
        lw_b = np.zeros((P, 2, LW_COLS // 2), dtype=np.float64)
        # col = 256*ks + 6*jj + m
        lw_b[:, :, :NM_PLANES * N_DC * NPC] = (
            L.transpose(0, 2, 1, 3).reshape(P, 2, NM_PLANES * N_DC * NPC))
        in_maps.append({
            "ep8": np.ascontiguousarray(ep_mask_embed[b]).astype(NP_F8),
            "q8": np.ascontiguousarray(query_mask_embed[b]).astype(NP_F8),
            "lw": lw_b.reshape(P, LW_COLS).astype(NP_F8),
        })
        counts.append((em.sum(), (1.0 - em).sum(), qm.sum(), (1.0 - qm).sum()))
    return in_maps, counts


def finalize(per_core, counts):
    """per_core: list of 8 arrays [2, 768] (epw|qw|qsqw) -> full (3,)."""
    pos = np.zeros(B)
    neg = np.zeros(B)
    for b in range(B):
        st = np.asarray(per_core[b]).astype(np.float64)
        n_pe, n_ne, n_pq, n_nq = counts[b]
        epw, qw, qsq = st[:, 0:C], st[:, C:2 * C], st[:, 2 * C:3 * C]
        pc = epw[0] / (n_pe + 0.1)
        ncen = epw[1] / (n_ne + 0.1)
        pn = qsq[0].sum() - 2.0 * (pc @ qw[0]) + n_pq * (pc @ pc)
        nn = qsq[1].sum() - 2.0 * (ncen @ qw[1]) + n_nq * (ncen @ ncen)
        pos[b] = pn / (max(n_pq, 1.0) * C) if n_pq > 0 else 0.0
        neg[b] = nn / (max(n_nq, 1.0) * C) if n_nq > 0 else 0.0
    return np.array(
        [(pos + neg).mean(), pos.mean(), neg.mean()], dtype=np.float32
    )


def kernel(ep_mask_embed, ep_mask, query_mask_embed, query_mask):
    ep_mask_embed = np.asarray(ep_mask_embed, dtype=np.float32)
    ep_mask = np.asarray(ep_mask, dtype=np.float32)
    query_mask_embed = np.asarray(query_mask_embed, dtype=np.float32)
    query_mask = np.asarray(query_mask, dtype=np.float32)

    nc = get_nc()
    in_maps, counts = make_in_maps(
        ep_mask_embed, ep_mask, query_mask_embed, query_mask)
    # First execution after device bring-up has been observed to return
    # garbage once; retry on non-finite results.
    for _ in range(3):
        res = run_bass_kernel_spmd(nc, in_maps, list(range(B)))
        result = finalize([r["out"] for r in res.results], counts)
        if np.all(np.isfinite(result)):
            return result
    return result
